# revision 8
# baseline (speedup 1.0000x reference)
"""Trainium2 Bass kernel for nn_CorrLoss: margin-ranking loss over a Gram matrix.

loss = mean_i relu( max_{j: t_j != t_i} corr[i,j] - min_{j: t_j == t_i} corr[i,j] + 40 )
with corr = feat @ feat.T, feat [4096, 512] f32, targets [4096] int.

Sharding: row-data-parallel over 8 NeuronCores. Each core computes its 512 rows
of corr via locT.T @ featT (featT replicated, 8MB/core) on the PE in float32r
(full-rate fp32), builds same/diff-class masks with tensor_scalar(is_equal) on
the DVE, and folds the masked row min/max into one tensor_tensor_reduce per
(chunk, direction): scr = corr +/- BIG*mask, then a plain row reduce. Per-row
ap/an [128, 4] go back to the host, which does relu(an-ap+40) and the mean.
"""
import sys
from contextlib import ExitStack

import numpy as np

sys.path.insert(0, "/opt/trn_rl_repo")

import concourse.bass as bass  # noqa: E402
from concourse import mybir  # noqa: E402
from concourse.bass_utils import run_bass_kernel_spmd  # noqa: E402

N_CORES = 8
N_ROWS = 4096
D = 512
M = N_ROWS // N_CORES   # 512 local rows
KT = D // 128           # 4
MT = M // 128           # 4
NCHUNK = 512
NT = N_ROWS // NCHUNK   # 8
MARGIN = 40.0
BIG = 1e30

_CACHE = {}


def _build():
    f32 = mybir.dt.float32
    f32r = mybir.dt.float32r
    op = mybir.AluOpType
    nc = bass.Bass("TRN2", target_bir_lowering=False, debug=False)
    fT = nc.declare_dram_parameter("fT", [D, N_ROWS], f32r, isOutput=False)
    locT = nc.declare_dram_parameter("locT", [D, M], f32r, isOutput=False)
    tall = nc.declare_dram_parameter("tall", [128, N_ROWS], f32, isOutput=False)
    tloc = nc.declare_dram_parameter("tloc", [128, MT], f32, isOutput=False)
    pl = nc.declare_dram_parameter("pl", [128, MT], f32, isOutput=True)
    apo = nc.declare_dram_parameter("apo", [128, MT], f32, isOutput=True)
    ano = nc.declare_dram_parameter("ano", [128, MT], f32, isOutput=True)

    with ExitStack() as ctx:
        fTs = ctx.enter_context(nc.sbuf_tensor("fTs", [128, KT * N_ROWS], f32r))
        locTs = ctx.enter_context(nc.sbuf_tensor("locTs", [128, KT * M], f32r))
        tall_sb = ctx.enter_context(nc.sbuf_tensor("tall_sb", [128, N_ROWS], f32))
        tloc_sb = ctx.enter_context(nc.sbuf_tensor("tloc_sb", [128, MT], f32))
        qb = ctx.enter_context(nc.sbuf_tensor("qb", [128, NT * NCHUNK], f32))
        sb = ctx.enter_context(nc.sbuf_tensor("sb", [128, NT * NCHUNK], f32))
        scr1 = ctx.enter_context(nc.sbuf_tensor("scr1", [128, NCHUNK], f32))
        scr2 = ctx.enter_context(nc.sbuf_tensor("scr2", [128, NCHUNK], f32))
        ap_acc = ctx.enter_context(nc.sbuf_tensor("ap_acc", [128, NT], f32))
        an_acc = ctx.enter_context(nc.sbuf_tensor("an_acc", [128, NT], f32))
        ap_fin = ctx.enter_context(nc.sbuf_tensor("ap_fin", [128, MT], f32))
        an_fin = ctx.enter_context(nc.sbuf_tensor("an_fin", [128, MT], f32))
        dcol = ctx.enter_context(nc.sbuf_tensor("dcol", [128, 1], f32))
        pl_sb = ctx.enter_context(nc.sbuf_tensor("pl_sb", [128, MT], f32))
        pt = [ctx.enter_context(nc.psum_tensor(f"pt{i}", [128, NCHUNK], f32))
              for i in range(4)]
        dma_in = ctx.enter_context(nc.semaphore("dma_in"))
        mm_sem = ctx.enter_context(nc.semaphore("mm_sem"))
        dve_sem = ctx.enter_context(nc.semaphore("dve_sem"))
        done_sem = ctx.enter_context(nc.semaphore("done_sem"))
        block = ctx.enter_context(nc.Block())

        @block.sync
        def _(sync):
            for k in range(KT):
                sync.dma_start(fTs[:, k * N_ROWS:(k + 1) * N_ROWS],
                               fT[k * 128:(k + 1) * 128, :]).then_inc(dma_in, 16)
                sync.dma_start(locTs[:, k * M:(k + 1) * M],
                               locT[k * 128:(k + 1) * 128, :]).then_inc(dma_in, 16)
            sync.dma_start(tall_sb[:], tall[:]).then_inc(dma_in, 16)
            sync.dma_start(tloc_sb[:], tloc[:]).then_inc(dma_in, 16)
            sync.wait_ge(done_sem, 1)
            sync.dma_start(pl[:], pl_sb[:]).then_inc(dma_in, 16)
            sync.dma_start(apo[:], ap_fin[:]).then_inc(dma_in, 16)
            sync.dma_start(ano[:], an_fin[:]).then_inc(dma_in, 16)
            sync.wait_ge(dma_in, 208)

        @block.tensor
        def _(tensor):
            tensor.wait_ge(dma_in, 128)  # fT + locT loaded
            for m in range(MT):
                for n in range(NT):
                    c = m * NT + n
                    b = c % 4
                    if c >= 4:
                        tensor.wait_ge(dve_sem, c - 3)
                    for k in range(KT):
                        mm = nc.tensor.matmul(
                            pt[b][:],
                            locTs[:, k * M + m * 128:k * M + (m + 1) * 128],
                            fTs[:, k * N_ROWS + n * NCHUNK:
                                k * N_ROWS + (n + 1) * NCHUNK],
                            start=(k == 0), stop=(k == KT - 1))
                        if k == KT - 1:
                            mm.then_inc(mm_sem, 1)

        @block.vector
        def _(vector):
            vector.wait_ge(dma_in, 160)  # tall + tloc loaded
            for m in range(MT):
                for n in range(NT):
                    cseg = tall_sb[:, n * NCHUNK:(n + 1) * NCHUNK]
                    nc.vector.tensor_scalar(
                        qb[:, n * NCHUNK:(n + 1) * NCHUNK], cseg,
                        tloc_sb[:, m:m + 1], BIG, op0=op.is_equal, op1=op.mult)
                    nc.vector.tensor_scalar(
                        sb[:, n * NCHUNK:(n + 1) * NCHUNK], cseg,
                        tloc_sb[:, m:m + 1], BIG, op0=op.not_equal, op1=op.mult)
                for n in range(NT):
                    c = m * NT + n
                    b = c % 4
                    vector.wait_ge(mm_sem, c + 1)
                    nc.vector.tensor_tensor(
                        scr1[:], pt[b][:], sb[:, n * NCHUNK:(n + 1) * NCHUNK],
                        op=op.add)
                    tt2 = nc.vector.tensor_tensor(
                        scr2[:], pt[b][:], qb[:, n * NCHUNK:(n + 1) * NCHUNK],
                        op=op.subtract)
                    tt2.then_inc(dve_sem, 1)
                    nc.vector.tensor_reduce(
                        ap_acc[:, n:n + 1], scr1[:],
                        axis=mybir.AxisListType.X, op=op.min)
                    nc.vector.tensor_reduce(
                        an_acc[:, n:n + 1], scr2[:],
                        axis=mybir.AxisListType.X, op=op.max)
                nc.vector.tensor_reduce(
                    ap_fin[:, m:m+1], ap_acc[:], axis=mybir.AxisListType.X, op=op.min)
                nc.vector.tensor_reduce(
                    an_fin[:, m:m+1], an_acc[:], axis=mybir.AxisListType.X, op=op.max)
                nc.vector.tensor_tensor(
                    dcol[:], an_fin[:, m:m+1], ap_fin[:, m:m+1], op=op.subtract)
                fin = nc.vector.tensor_scalar(
                    pl_sb[:, m:m + 1], dcol[:], MARGIN, 0.0,
                    op0=op.add, op1=op.max)
                if m == MT - 1:
                    fin.then_inc(done_sem, 1)
    return nc


def kernel(feat: np.ndarray, targets: np.ndarray) -> np.ndarray:
    feat = np.ascontiguousarray(np.asarray(feat, dtype=np.float32))
    tg = np.asarray(targets)

    fT = np.ascontiguousarray(feat.T)                       # [512, 4096]
    tgf = tg.astype(np.float32)
    tall = np.ascontiguousarray(np.broadcast_to(tgf[None, :], (128, N_ROWS)))

    if "nc" not in _CACHE:
        _CACHE["nc"] = _build()
    nc = _CACHE["nc"]

    in_maps = []
    for c in range(N_CORES):
        locT = np.ascontiguousarray(fT[:, c * M:(c + 1) * M])
        tloc = np.ascontiguousarray(
            tgf[c * M:(c + 1) * M].reshape(MT, 128).T)       # [128, MT]
        in_maps.append({"fT": fT, "locT": locT, "tall": tall, "tloc": tloc})

    res = run_bass_kernel_spmd(nc, in_maps, list(range(N_CORES)))
    total = 0.0
    for c in range(N_CORES):
        ap = res.results[c]["apo"].astype(np.float64)
        an = res.results[c]["ano"].astype(np.float64)
        total += np.maximum(an - ap + MARGIN, 0.0).sum()
    return np.asarray(np.float32(total / N_ROWS))
